# revision 1
# baseline (speedup 1.0000x reference)
"""MixtureOfExpertsTreeEnsemble Trainium2 kernel (8-core SPMD, batch data-parallel).

Math (per batch row b, tree t):
  g[b,n,t] = sigmoid(x[b] @ W[n,:,t] + bias[n,t])          63 internal nodes
  p[b,l,t] = prod of g / (1-g) along root->leaf path        64 leaves
  w[l,d,t] = leaf_weight[l,d,t] * softmax_t(gates[l,d,t])
  out[b,d] = sum_{l,t} p[b,l,t] * w[l,d,t]

Sharding: batch 4096 -> 8 cores x 512 rows; node weights / leaf tables are
replicated (small).  No collectives; host concatenates the per-core outputs.

Device-side structure (per core):
  * all streamed operands are bf16 (the gates are bf16 downstream anyway, so
    f32 logits precision would be wasted); PSUM accumulation stays f32
  * phase A (PE):   logits as [b_tile(128), (node,tree)] bf16 matmuls.
    Loop is (btile-pair, chunk): each (node,tree) chunk is consumed as soon
    as its DMA lands, and one [128,1008] 2-bank PSUM tile serves two batch
    tiles -> half the sigmoid instructions on ACT
  * phase B (DVE):  leaf path probabilities by level doubling in a *block*
    layout (children stored [left | right]) so every op is contiguous and
    bf16 (DVE 2x mode); host pre-permutes node order (bit-reversal within
    each level) and the leaf tables to match
  * phase C (PE):   p transposed to [(leaf,tree), b] bf16 chunks, 4 chunks
    per PSUM bank, one DVE copy per bank
  * phase 0:        w = leaf_weight * softmax(gates) with (l,d) on partitions
    and t free: exp on ACT, reduce on DVE, the 64 normalize ops on the idle
    Pool engine; PE transposes are emitted after the batch loop so they
    never block phase A on the leaf-table DMAs
  * phase D (PE):   out_T[d,b] = sum_chunks w_T.T @ p_T, host transposes back
  * DMA: weight matrix on the SP HW-DGE ring; x / leaf tables / output on the
    ACT ring so the two streams overlap
"""

import sys

sys.path.insert(0, "/opt/trn_rl_repo")

import ml_dtypes
import numpy as np

BF16 = np.dtype(ml_dtypes.bfloat16)

MAX_DEPTH = 6
NUM_TREES = 64
LEAF_DIMS = 128
D_IN = 512
BATCH = 4096
N_INTERNAL = 63
N_LEAVES = 64
N_CORES = 8
BS = BATCH // N_CORES          # 512 batch rows per core
KT = D_IN // 128               # 4 contraction tiles
NT = N_INTERNAL * NUM_TREES    # 4032 (node,tree) pairs
NCHUNK = 8
CHUNK = NT // NCHUNK           # 504
NBT = BS // 128                # 4 batch tiles per core
NPT = N_LEAVES * NUM_TREES // 128  # 32 transpose chunks of (leaf,tree)


def _bitrev(x: int, bits: int) -> int:
    r = 0
    for _ in range(bits):
        r = (r << 1) | (x & 1)
        x >>= 1
    return r


# block-recursion orderings (see module docstring)
_NODES_PERM = np.array(
    [(2**lvl - 1) + _bitrev(j, lvl) for lvl in range(MAX_DEPTH) for j in range(2**lvl)]
)
_LEAF_PERM = np.array([_bitrev(j, MAX_DEPTH) for j in range(N_LEAVES)])

_BUILT = {}


def _build(use_bias: bool):
    """Build + finalize the per-core Bass program."""
    import concourse.bacc as bacc
    import concourse.tile as tile
    from concourse import mybir
    from concourse.masks import make_identity

    f32 = mybir.dt.float32
    f32r = mybir.dt.float32r
    bf16 = mybir.dt.bfloat16
    AF = mybir.ActivationFunctionType
    AX = mybir.AxisListType
    MUL = mybir.AluOpType.mult

    nc = bacc.Bacc("TRN2", target_bir_lowering=False, debug=False)

    xT = nc.dram_tensor("xT", [KT, 128, BS], bf16, kind="ExternalInput")
    Wf = nc.dram_tensor("Wf", [KT, 128, NT], bf16, kind="ExternalInput")
    # leaf tables host-transposed to [d, (leaf, tree)] for contiguous DMA
    gt = nc.dram_tensor("gt", [LEAF_DIMS, N_LEAVES * NUM_TREES], bf16, kind="ExternalInput")
    lwt = nc.dram_tensor("lwt", [LEAF_DIMS, N_LEAVES * NUM_TREES], bf16, kind="ExternalInput")
    if use_bias:
        bias = nc.dram_tensor("bias", [1, NT], bf16, kind="ExternalInput")
    outT = nc.dram_tensor("outT", [LEAF_DIMS, BS], f32, kind="ExternalOutput")

    with tile.TileContext(nc) as tc:
        with tc.tile_pool(name="const", bufs=1) as cpool, \
             tc.tile_pool(name="wts", bufs=1) as wpool, \
             tc.tile_pool(name="psA", bufs=4, space="PSUM") as psA, \
             tc.tile_pool(name="psT", bufs=3, space="PSUM") as psT, \
             tc.tile_pool(name="psO", bufs=1, space="PSUM") as psO:

            ident = cpool.tile([128, 128], bf16, tag="ident")
            make_identity(nc, ident[:])

            # ---- input DMAs.  x + leaf tables on the ACT HW-DGE ring, the
            # (bigger) weight matrix on the SP ring, so they overlap. ----
            xk = []
            for k in range(KT):
                t = wpool.tile([128, BS], bf16, tag=f"xk{k}", name=f"xk{k}")
                nc.scalar.dma_start(t[:], xT[k, :, :])
                xk.append(t)

            wk = [wpool.tile([128, NT], bf16, tag=f"wk{k}", name=f"wk{k}") for k in range(KT)]
            for m in range(NCHUNK // 2):
                for k in range(KT):
                    nc.sync.dma_start(
                        wk[k][:, m * 2 * CHUNK:(m + 1) * 2 * CHUNK],
                        Wf[k, :, m * 2 * CHUNK:(m + 1) * 2 * CHUNK],
                    )
            if use_bias:
                bias_sb = cpool.tile([1, NT], bf16, tag="bias")
                nc.sync.dma_start(bias_sb[:], bias[:, :])
                ones1 = cpool.tile([1, 128], bf16, tag="ones1")
                nc.gpsimd.memset(ones1[:], 1.0)

            wsm = wpool.tile([128, N_LEAVES, NUM_TREES], bf16, tag="wsm")
            gtile = wpool.tile([128, N_LEAVES, NUM_TREES], bf16, tag="gtile")
            nc.sync.dma_start(gtile[:], gt[:, :].rearrange("d (l t) -> d l t", t=NUM_TREES))
            lwtile = wpool.tile([128, N_LEAVES, NUM_TREES], bf16, tag="lwtile")
            nc.sync.dma_start(lwtile[:], lwt[:, :].rearrange("d (l t) -> d l t", t=NUM_TREES))

            def emit_phase0_compute():
                # w = leaf_weight * softmax(gates): exp on ACT, reduce on DVE,
                # normalize on the idle Pool engine (PE transposes deferred)
                nc.scalar.activation(gtile[:], gtile[:], AF.Exp)
                ehalf = cpool.tile([128, N_LEAVES, NUM_TREES // 2], bf16, tag="ehalf")
                nc.vector.tensor_add(ehalf[:], gtile[:, :, 0:NUM_TREES // 2],
                                     gtile[:, :, NUM_TREES // 2:NUM_TREES])
                equar = cpool.tile([128, N_LEAVES, NUM_TREES // 4], bf16, tag="equar")
                nc.vector.tensor_add(equar[:], ehalf[:, :, 0:NUM_TREES // 4],
                                     ehalf[:, :, NUM_TREES // 4:NUM_TREES // 2])
                s = cpool.tile([128, N_LEAVES], f32, tag="s")
                nc.vector.reduce_sum(s[:], equar[:], axis=AX.X)
                r = cpool.tile([128, N_LEAVES], f32, tag="r")
                nc.vector.reciprocal(r[:], s[:])
                for l in range(N_LEAVES):
                    nc.vector.scalar_tensor_tensor(
                        wsm[:, l, :], gtile[:, l, :], r[:, l:l + 1], lwtile[:, l, :],
                        op0=MUL, op1=MUL,
                    )

            # ---- main loop: batch-tile pairs ----
            with tc.tile_pool(name="gp", bufs=1) as gpool, \
                 tc.tile_pool(name="pp", bufs=2) as ppool, \
                 tc.tile_pool(name="pfp", bufs=2) as pfpool, \
                 tc.tile_pool(name="ptp", bufs=1) as pTpool, \
                 tc.tile_pool(name="outp", bufs=1) as outpool:

                out_ps = psO.tile([LEAF_DIMS, BS], f32, tag="out_ps")
                out_sb = outpool.tile([LEAF_DIMS, BS], f32, tag="out_sb")

                # PE warm-up: dummy transposes into the (not yet used) output
                # PSUM bank while the first weight DMAs are in flight, so the
                # HAM clock gate is released before phase A starts
                ident32 = cpool.tile([128, 128], f32, tag="ident32")
                make_identity(nc, ident32[:])
                for _wi in range(10):
                    nc.tensor.transpose(out_ps[:, 0:128], ident32[:], ident32[:])

                def emit_phaseD_slice(i, width=1):
                    bsl = slice(i * 128, (i + width) * 128)
                    for j in range(NPT):
                        nc.tensor.matmul(out_ps[:, bsl],
                                         wT_all[:, j // 4, (j % 4) * 128:(j % 4 + 1) * 128],
                                         pT_all[:, j, bsl],
                                         start=(j == 0), stop=(j == NPT - 1))
                    nc.vector.tensor_copy(out_sb[:, bsl], out_ps[:, bsl])
                    nc.sync.dma_start(outT[:, bsl], out_sb[:, bsl])

                # pT_all[:, j, :] = chunk j of p_T, [(leaf,tree)(128), b(512)]
                pT_all = pTpool.tile([128, NPT, BS], bf16, tag="pT")
                # g_all[:, i, :] = sigmoid gates for batch tile i
                g_all = gpool.tile([128, NBT, NT], bf16, tag="g")
                wT_all = wpool.tile([128, NPT // 4, 512], bf16, tag="wT")

                for pair in range(NBT // 2):
                    i0 = 2 * pair
                    # phase A: one 2-bank PSUM tile serves both batch tiles of
                    # the pair; chunks consumed in DMA arrival order
                    for q in range(2):
                        bsl = slice((i0 + q) * 128, (i0 + q + 1) * 128)
                        for n in range(NCHUNK):
                            csl = slice(n * CHUNK, (n + 1) * CHUNK)
                            lg = psA.tile([128, CHUNK], f32, tag="lg")
                            for k in range(KT):
                                nc.tensor.matmul(
                                    lg[:], xk[k][:, bsl], wk[k][:, csl],
                                    start=(k == 0),
                                    stop=(k == KT - 1 and not use_bias),
                                )
                            if use_bias:
                                nc.tensor.matmul(
                                    lg[:], ones1[:], bias_sb[:, csl],
                                    start=False, stop=True,
                                )
                            nc.scalar.activation(g_all[:, i0 + q, csl], lg[:], AF.Sigmoid)

                    if pair == 1:
                        # phase 0 PE part here: wsm is ready by now and this
                        # keeps the post-loop PE tail short
                        for jj in range(NPT // 4):
                            tp4 = psT.tile([128, 512], bf16, tag="tp")
                            for qq in range(4):
                                j = 4 * jj + qq
                                nc.tensor.transpose(
                                    tp4[:, qq * 128:(qq + 1) * 128],
                                    wsm[:, 2 * j:2 * j + 2, :], ident[:])
                            nc.scalar.copy(wT_all[:, jj, :], tp4[:])

                        emit_phaseD_slice(0, width=2)

                    for q in range(2):
                        i = i0 + q
                        bsl = slice(i * 128, (i + 1) * 128)
                        # phase B: block-layout level doubling (DVE, 2x mode)
                        pa = ppool.tile([128, 2048], bf16, tag="pa")
                        pb = ppool.tile([128, 2048], bf16, tag="pb")
                        pf = pfpool.tile([128, 4096], bf16, tag="pf")
                        # level 0: p = [g0 | 1-g0]
                        nc.vector.tensor_copy(pa[:, 0:64], g_all[:, i, 0:64])
                        nc.scalar.activation(pa[:, 64:128], g_all[:, i, 0:64],
                                             AF.Copy, bias=1.0, scale=-1.0)
                        cur, other = pa, pb
                        for lvl in range(1, MAX_DEPTH):
                            h = (2 ** lvl) * 64
                            off = (2 ** lvl - 1) * 64
                            dst = pf if lvl == MAX_DEPTH - 1 else other
                            if lvl == MAX_DEPTH - 1:
                                # half-split so the first transpose chunks can
                                # start while the second half still computes
                                hh = h // 2
                                nc.vector.tensor_mul(dst[:, 0:hh], cur[:, 0:hh],
                                                     g_all[:, i, off:off + hh])
                                nc.vector.tensor_sub(dst[:, h:h + hh], cur[:, 0:hh],
                                                     dst[:, 0:hh])
                                nc.vector.tensor_mul(dst[:, hh:h], cur[:, hh:h],
                                                     g_all[:, i, off + hh:off + h])
                                nc.vector.tensor_sub(dst[:, h + hh:2 * h], cur[:, hh:h],
                                                     dst[:, hh:h])
                            else:
                                nc.vector.tensor_mul(dst[:, 0:h], cur[:, 0:h],
                                                     g_all[:, i, off:off + h])
                                nc.vector.tensor_sub(dst[:, h:2 * h], cur[:, 0:h],
                                                     dst[:, 0:h])
                            cur, other = dst, cur

                        # phase C: transpose p -> [(leaf,tree), b] bf16;
                        # 4 chunks share one PSUM bank, one DVE copy per bank
                        for jj in range(NPT // 4):
                            tp4 = psT.tile([128, 512], bf16, tag="tp")
                            for qq in range(4):
                                j = 4 * jj + qq
                                nc.tensor.transpose(
                                    tp4[:, qq * 128:(qq + 1) * 128],
                                    pf[:, j * 128:(j + 1) * 128], ident[:])
                            mod = 2 if pair == 1 else 3
                            if jj % mod == mod - 1:
                                nc.scalar.copy(
                                    pT_all[:, 4 * jj:4 * jj + 4, bsl], tp4[:])
                            else:
                                nc.vector.tensor_copy(
                                    pT_all[:, 4 * jj:4 * jj + 4, bsl], tp4[:])

                        if pair == 1:
                            emit_phaseD_slice(i)

                    if pair == 0:
                        emit_phase0_compute()



    nc.finalize()
    return nc


def _get_nc(use_bias: bool):
    if use_bias not in _BUILT:
        _BUILT[use_bias] = _build(use_bias)
    return _BUILT[use_bias]


def _make_in_maps(x, W, b, leaf_weight, gates):
    x = np.ascontiguousarray(np.asarray(x, dtype=np.float32))
    W = np.asarray(W, dtype=np.float32)
    b = np.asarray(b, dtype=np.float32)
    leaf_weight = np.asarray(leaf_weight, dtype=np.float32)
    gates = np.asarray(gates, dtype=np.float32)

    use_bias = bool(np.any(b))
    # host-side layout prep (permutations / transposes / bf16 cast)
    Wp = W[_NODES_PERM]                                   # [63, 512, 64]
    Wf = np.ascontiguousarray(
        Wp.transpose(1, 0, 2).reshape(KT, 128, NT).astype(BF16))
    # leaf tables -> [d, (leaf, tree)] so the DMA is contiguous per partition
    gt = np.ascontiguousarray(
        gates[_LEAF_PERM].transpose(1, 0, 2).reshape(LEAF_DIMS, -1).astype(BF16))
    lwt = np.ascontiguousarray(
        leaf_weight[_LEAF_PERM].transpose(1, 0, 2).reshape(LEAF_DIMS, -1).astype(BF16))
    if use_bias:
        bias = np.ascontiguousarray(b[_NODES_PERM].reshape(1, NT).astype(BF16))

    in_maps = []
    for c in range(N_CORES):
        xs = x[c * BS:(c + 1) * BS]                       # [512, 512]
        xTc = np.ascontiguousarray(xs.T.reshape(KT, 128, BS).astype(BF16))
        m = {"xT": xTc, "Wf": Wf, "gt": gt, "lwt": lwt}
        if use_bias:
            m["bias"] = bias
        in_maps.append(m)
    return use_bias, in_maps


def kernel(x, W, b, leaf_weight, gates):
    from concourse.bass_utils import run_bass_kernel_spmd

    use_bias, in_maps = _make_in_maps(x, W, b, leaf_weight, gates)
    nc = _get_nc(use_bias)

    res = run_bass_kernel_spmd(nc, in_maps, core_ids=list(range(N_CORES)))
    out = np.empty((BATCH, LEAF_DIMS), dtype=np.float32)
    for c in range(N_CORES):
        out[c * BS:(c + 1) * BS] = res.results[c]["outT"].T
    return out



# revision 10
# speedup vs baseline: 1.0489x; 1.0489x over previous
"""MixtureOfExpertsTreeEnsemble Trainium2 kernel (8-core SPMD, batch data-parallel).

Math (per batch row b, tree t):
  g[b,n,t] = sigmoid(x[b] @ W[n,:,t] + bias[n,t])          63 internal nodes
  p[b,l,t] = prod of g / (1-g) along root->leaf path        64 leaves
  w[l,d,t] = leaf_weight[l,d,t] * softmax_t(gates[l,d,t])
  out[b,d] = sum_{l,t} p[b,l,t] * w[l,d,t]

Sharding: batch 4096 -> 8 cores x 512 rows; node weights / leaf tables are
replicated.  No collectives; host concatenates the per-core outputs.

Device-side design (per core), all in a TRANSPOSED [(node,tree), batch]
layout so the path products need no PE transposes at all:

  * phase A (PE):   z^T[(n,t), b] tiles; nodes level-major, within-level
    bit-reversed ("block" order), trees inner.  Levels 0-4 (31 nodes) in
    bf16; level 5 (32 nodes) in fp8(e4m3) DoubleRow matmuls (2x PE rate;
    measured end-to-end rel-err ~1.3e-2 < 2e-2).  Node 0's bank holds
    [W0 | -W0] so one sigmoid op produces p1 = [g0 | 1-g0] for free.
  * sigmoid (ACT):  per 2-bank PSUM tile, writing per-level g tensors
    [128 part=(node,tree), 512 b].  exp for the softmax runs FIRST so the
    ACT function table loads only twice (exp set -> sigmoid set).
  * phase B (DVE):  level doubling entirely along partitions:
    p_{l+1} = [p_l * g_l | p_l * (1-g_l)]; the right half uses a fresh
    product with h=1-g (tensor_scalar) instead of a subtract to avoid a
    double-rounding that costs ~4e-3 of accuracy.
  * leaves:         only the LEFT leaf products pLL = p5*g5 materialize.
    The right-leaf term is folded into phase D algebraically:
      out = sum_j (wTL_j - wTR_j)^T pLL_j + sum_j wTR_j^T p5_j
    which deletes 32 DVE subtract ops at zero PE cost.
  * phase 0:        w = leaf_weight * softmax(gates): exp on ACT (bf16),
    tree-sum + recip on DVE, e*r broadcast on the otherwise idle Pool
    engine, *leaf_weight on DVE; PE transposes w -> [(leaf,tree), d].
  * phase D (PE):   out^T[d,b] accumulated over 32 [(l,t),*] chunks; the
    wT transposes and the first 16 chunk matmuls interleave into phase
    A-fp8's ACT-paced PE gaps.
  * DMA: weights + x on the SP HW-DGE ring, leaf tables on the ACT ring.
"""

import sys

sys.path.insert(0, "/opt/trn_rl_repo")

import ml_dtypes
import numpy as np

BF16 = np.dtype(ml_dtypes.bfloat16)
FP8 = np.dtype(ml_dtypes.float8_e4m3)

MAX_DEPTH = 6
NUM_TREES = 64
LEAF_DIMS = 128
D_IN = 512
BATCH = 4096
N_INTERNAL = 63
N_LEAVES = 64
N_CORES = 8
BS = BATCH // N_CORES          # 512 batch rows per core
KT = D_IN // 128               # 4 contraction tiles
NBF = 16                       # bf16 banks: lvl0+- 1, lvl1 1, lvl2 2, lvl3 4, lvl4 8
NF8 = 8                        # fp8 4-node groups (level 5: 32 nodes)


def _bitrev(x: int, bits: int) -> int:
    r = 0
    for _ in range(bits):
        r = (r << 1) | (x & 1)
        x >>= 1
    return r


# level-major, within-level bit-reversed (block recursion) node order
_NODES_PERM = np.array(
    [(2**lvl - 1) + _bitrev(j, lvl) for lvl in range(MAX_DEPTH) for j in range(2**lvl)]
)
_LEAF_PERM = np.array([_bitrev(j, MAX_DEPTH) for j in range(N_LEAVES)])

_BUILT = {}


def _build(use_bias: bool):
    import concourse.bacc as bacc
    import concourse.tile as tile
    from concourse import mybir
    from concourse.masks import make_identity

    f32 = mybir.dt.float32
    bf16 = mybir.dt.bfloat16
    fp8 = mybir.dt.float8e4
    AF = mybir.ActivationFunctionType
    AX = mybir.AxisListType
    MUL = mybir.AluOpType.mult
    ADD = mybir.AluOpType.add
    DR = mybir.MatmulPerfMode.DoubleRow

    nc = bacc.Bacc("TRN2", target_bir_lowering=False, debug=False)

    xT = nc.dram_tensor("xT", [128, KT, BS], bf16, kind="ExternalInput")
    x8 = nc.dram_tensor("x8", [128, 2, 2, BS], fp8, kind="ExternalInput")
    Wbf = nc.dram_tensor("Wbf", [NBF, 128, KT, 128], bf16, kind="ExternalInput")
    W8 = nc.dram_tensor("W8", [2 * NF8, 128, 2, 2, 128], fp8, kind="ExternalInput")
    gt = nc.dram_tensor("gt", [LEAF_DIMS, N_LEAVES, NUM_TREES], bf16, kind="ExternalInput")
    lwt = nc.dram_tensor("lwt", [LEAF_DIMS, N_LEAVES, NUM_TREES], bf16, kind="ExternalInput")
    if use_bias:
        biasd = nc.dram_tensor("biasd", [128, NBF + 2 * NF8], f32, kind="ExternalInput")
    outT = nc.dram_tensor("outT", [LEAF_DIMS, BS], f32, kind="ExternalOutput")

    with tile.TileContext(nc) as tc:
        with tc.tile_pool(name="const", bufs=1) as cpool, \
             tc.tile_pool(name="wts", bufs=1) as wpool, \
             tc.tile_pool(name="psA", bufs=3, space="PSUM") as psA, \
             tc.tile_pool(name="psT", bufs=1, space="PSUM") as psT, \
             tc.tile_pool(name="psO", bufs=1, space="PSUM") as psO:

            ident = cpool.tile([128, 128], bf16, tag="ident")
            make_identity(nc, ident[:])

            # ---- input DMAs.  leaf tables on the ACT ring; x + weights on
            # the SP ring so the streams overlap. ----
            gtile = wpool.tile([128, N_LEAVES, NUM_TREES], bf16, tag="gtile")
            nc.scalar.dma_start(gtile[:, 0:32, :], gt[:, 0:32, :])
            nc.scalar.dma_start(gtile[:, 32:64, :], gt[:, 32:64, :])
            lwtile = wpool.tile([128, N_LEAVES, NUM_TREES], bf16, tag="lwtile")
            nc.scalar.dma_start(lwtile[:], lwt[:, :, :])

            xk = wpool.tile([128, KT, BS], bf16, tag="xk")
            for k in range(KT):
                nc.sync.dma_start(xk[:, k, :], xT[:, k, :])
            x8sb = wpool.tile([128, 2, 2, BS], fp8, tag="x8sb")
            nc.sync.dma_start(x8sb[:], x8[:, :, :, :])
            wbf_sb = wpool.tile([128, NBF, KT, 128], bf16, tag="wbf")
            for m in range(NBF):
                nc.sync.dma_start(wbf_sb[:, m, :, :], Wbf[m, :, :, :])
            w8_sb = wpool.tile([128, 2 * NF8, 2, 2, 128], fp8, tag="w8")
            for q in range(2 * NF8):
                nc.sync.dma_start(w8_sb[:, q, :, :, :], W8[q, :, :, :, :])
            if use_bias:
                bias_sb = cpool.tile([128, NBF + 2 * NF8], f32, tag="bias")
                nc.sync.dma_start(bias_sb[:], biasd[:, :])

            # ---- SBUF state ----
            g1 = wpool.tile([128, BS], bf16, tag="g1")
            g2 = wpool.tile([128, 2, BS], bf16, tag="g2")
            g3 = wpool.tile([128, 4, BS], bf16, tag="g3")
            g4 = wpool.tile([128, 8, BS], bf16, tag="g4")
            g5 = wpool.tile([128, 16, BS], bf16, tag="g5")
            h1 = wpool.tile([128, BS], bf16, tag="h1")
            h2 = wpool.tile([128, 2, BS], bf16, tag="h2")
            h3 = wpool.tile([128, 4, BS], bf16, tag="h3")
            h4 = wpool.tile([128, 8, BS], bf16, tag="h4")
            p1 = wpool.tile([128, BS], bf16, tag="p1")
            p2 = wpool.tile([128, 2, BS], bf16, tag="p2")
            p3 = wpool.tile([128, 4, BS], bf16, tag="p3")
            p4 = wpool.tile([128, 8, BS], bf16, tag="p4")
            p5 = wpool.tile([128, 16, BS], bf16, tag="p5")
            pLL = wpool.tile([128, 16, BS], bf16, tag="pLL")
            s_t = cpool.tile([128, N_LEAVES], bf16, tag="s_t")
            r_t = cpool.tile([128, N_LEAVES], bf16, tag="r_t")
            wsmt = wpool.tile([128, N_LEAVES, NUM_TREES], bf16, tag="wsmt")
            wTall = wpool.tile([128, 32, 128], bf16, tag="wTall")
            wd = wpool.tile([128, 16, 128], bf16, tag="wd")
            out_sb = wpool.tile([LEAF_DIMS, BS], f32, tag="out_sb")

            glv = [None, g1, g2, g3, g4, g5]
            hlv = [None, h1, h2, h3, h4]
            plv = [None, p1, p2, p3, p4, p5]

            # ---- PE warm-up: release the clock gate / ramp p-state while
            # the first DMAs land ----
            warm = psT.tile([128, 4, 128], bf16, tag="tp")
            for _ in range(14):
                nc.tensor.transpose(warm[:, 0, :], ident[:], ident[:])

            # ---- ACT: softmax exp first (so the exp table load replaces the
            # initial sigmoid load; one switch to sigmoid afterwards) ----
            nc.scalar.activation(gtile[:, 0:32, :], gtile[:, 0:32, :], AF.Exp)
            nc.scalar.activation(gtile[:, 32:64, :], gtile[:, 32:64, :], AF.Exp)

            # ---- DVE/Pool softmax chain (emitted early; deps gate it) ----
            # tree-sums + recips per half on DVE, then en = e*r on Pool.
            # Slices pair the L (beta<32) and matching R (beta>=32) ranges so
            # each wT transpose group's inputs complete together; the final
            # wsm = en*lw DVE ops are emitted later (interleaved into phase
            # A) so they don't block the in-order DVE path-product chain.
            with nc.allow_low_precision(reason="softmax denom in bf16: validated "
                                        "end-to-end rel-err impact < 5e-4"):
                for hh in range(2):
                    sl = slice(32 * hh, 32 * (hh + 1))
                    nc.vector.reduce_sum(s_t[:, sl], gtile[:, sl, :], axis=AX.X)
                    nc.vector.reciprocal(r_t[:, sl], s_t[:, sl])
            wsm_slices = [slice(0, 16), slice(32, 48), slice(16, 32), slice(48, 64)]
            for sl in wsm_slices:
                rb = r_t[:, sl, None].broadcast_to((128, 16, NUM_TREES))
                nc.gpsimd.tensor_tensor(gtile[:, sl, :], gtile[:, sl, :], rb, op=MUL)

            def emit_wsm(idx):
                sl = wsm_slices[idx]
                nc.vector.tensor_mul(wsmt[:, sl, :], gtile[:, sl, :], lwtile[:, sl, :])

            # ---- helpers ----
            def sigmoid_op(src, dst, bias_col=None):
                if use_bias:
                    nc.scalar.activation(dst, src, AF.Sigmoid,
                                         bias=bias_sb[:, bias_col:bias_col + 1])
                else:
                    nc.scalar.activation(dst, src, AF.Sigmoid)

            def emit_bf16_tile(ti):
                """psA tile covering bf16 banks 2ti, 2ti+1 -> g tensors."""
                za = psA.tile([128, 2, BS], f32, tag="za")
                for hh in range(2):
                    m = 2 * ti + hh
                    for k in range(KT):
                        nc.tensor.matmul(za[:, hh, :], wbf_sb[:, m, k, :],
                                         xk[:, k, :], start=(k == 0), stop=(k == KT - 1))
                # sigmoid destinations
                if ti == 0:
                    sigmoid_op(za[:, 0, :], p1[:], 0)
                    sigmoid_op(za[:, 1, :], g1[:], 1)
                elif ti == 1:
                    if use_bias:
                        sigmoid_op(za[:, 0, :], g2[:, 0, :], 2)
                        sigmoid_op(za[:, 1, :], g2[:, 1, :], 3)
                    else:
                        sigmoid_op(za[:, :, :], g2[:, 0:2, :])
                else:
                    lvl = 3 if ti < 4 else 4
                    goff = 2 * (ti - 2) if ti < 4 else 2 * (ti - 4)
                    gdst = glv[lvl]
                    if use_bias:
                        sigmoid_op(za[:, 0, :], gdst[:, goff, :], 2 * ti)
                        sigmoid_op(za[:, 1, :], gdst[:, goff + 1, :], 2 * ti + 1)
                    else:
                        sigmoid_op(za[:, :, :], gdst[:, goff:goff + 2, :])

            def emit_fp8_round(r):
                """4 level-5 nodes (banks 2r, 2r+1) in fp8 DoubleRow -> g5."""
                za = psA.tile([128, 2, BS], f32, tag="za")
                for hh in range(2):
                    c = 2 * r + hh
                    for bh in range(2):
                        for kp in range(2):
                            nc.tensor.matmul(
                                za[:, hh, bh * 256:(bh + 1) * 256],
                                w8_sb[:, c, kp, :, :],
                                x8sb[:, kp, :, bh * 256:(bh + 1) * 256],
                                start=(kp == 0), stop=(kp == 1), perf_mode=DR)
                if use_bias:
                    sigmoid_op(za[:, 0, :], g5[:, 2 * r, :], NBF + 2 * r)
                    sigmoid_op(za[:, 1, :], g5[:, 2 * r + 1, :], NBF + 2 * r + 1)
                else:
                    sigmoid_op(za[:, :, :], g5[:, 2 * r:2 * r + 2, :])
                # DVE: left-leaf products for these two chunks
                csl = slice(2 * r, 2 * r + 2)
                nc.vector.tensor_mul(pLL[:, csl, :], p5[:, csl, :], g5[:, csl, :])

            def emit_level_products(lvl, coff, n):
                """p_{lvl+1} chunks [coff, coff+n) from p_lvl, g_lvl, h_lvl."""
                g, h, p, pn = glv[lvl], hlv[lvl], plv[lvl], plv[lvl + 1]
                half = pn.shape[1] // 2 if lvl > 1 else 1
                if lvl == 1:
                    nc.vector.tensor_scalar(h[:], g[:], -1.0, 1.0, op0=MUL, op1=ADD)
                    nc.vector.tensor_mul(pn[:, 0, :], p[:], g[:])
                    nc.vector.tensor_mul(pn[:, 1, :], p[:], h[:])
                else:
                    sl = slice(coff, coff + n)
                    slR = slice(half + coff, half + coff + n)
                    nc.vector.tensor_scalar(h[:, sl, :], g[:, sl, :], -1.0, 1.0,
                                            op0=MUL, op1=ADD)
                    nc.vector.tensor_mul(pn[:, sl, :], p[:, sl, :], g[:, sl, :])
                    nc.vector.tensor_mul(pn[:, slR, :], p[:, sl, :], h[:, sl, :])

            def emit_wT_group(gidx):
                """Transpose wsm chunks (2g, 2g+1, 16+2g, 16+2g+1) and build wd."""
                tp = psT.tile([128, 4, 128], bf16, tag="tp")
                chunks = [2 * gidx, 2 * gidx + 1, 16 + 2 * gidx, 16 + 2 * gidx + 1]
                for qi, c in enumerate(chunks):
                    nc.tensor.transpose(tp[:, qi, :], wsmt[:, 2 * c:2 * c + 2, :], ident[:])
                nc.vector.tensor_copy(wTall[:, 2 * gidx:2 * gidx + 2, :], tp[:, 0:2, :])
                nc.vector.tensor_copy(wTall[:, 16 + 2 * gidx:16 + 2 * gidx + 2, :],
                                      tp[:, 2:4, :])
                nc.vector.tensor_sub(wd[:, 2 * gidx:2 * gidx + 2, :],
                                     wTall[:, 2 * gidx:2 * gidx + 2, :],
                                     wTall[:, 16 + 2 * gidx:16 + 2 * gidx + 2, :])

            out_ps = psO.tile([LEAF_DIMS, BS], f32, tag="out_ps")
            dcount = [0]

            def emit_D(stationary, moving_chunk, moving):
                nc.tensor.matmul(out_ps[:], stationary, moving[:, moving_chunk, :],
                                 start=(dcount[0] == 0), stop=(dcount[0] == 31))
                dcount[0] += 1

            # ---- phase A bf16 (levels 0-4) with phase-B DVE ops interleaved ----
            emit_bf16_tile(0)                      # p1, g1
            emit_bf16_tile(1)                      # g2
            emit_level_products(1, 0, 1)           # p2
            emit_bf16_tile(2)                      # g3[0:2]
            emit_level_products(2, 0, 2)           # p3 (needs g2 only)
            emit_bf16_tile(3)                      # g3[2:4]
            emit_level_products(3, 0, 2)           # p4 chunks 0:2 / 4:6
            for i in range(4):                     # g4 tiles
                emit_bf16_tile(4 + i)
                if i == 0:
                    emit_level_products(3, 2, 2)   # rest of p4
                    emit_wsm(0)                    # enables wT groups 0-3
                    emit_wsm(1)
                emit_level_products(4, 2 * i, 2)   # p5 per g4 pair
            emit_wsm(2)                            # enables wT groups 4-7
            emit_wsm(3)

            # ---- phase A fp8 (level 5) + wT transposes + phase D interleave ----
            emit_fp8_round(0)
            emit_fp8_round(1)
            emit_fp8_round(2)
            emit_wT_group(0)
            emit_fp8_round(3)
            emit_wT_group(1)
            emit_fp8_round(4)
            emit_wT_group(2)
            for c in range(0, 4):
                emit_D(wTall[:, 16 + c, :], c, p5)
            emit_fp8_round(5)
            emit_wT_group(3)
            for c in range(4, 8):
                emit_D(wTall[:, 16 + c, :], c, p5)
            emit_fp8_round(6)
            emit_wT_group(4)
            emit_wT_group(5)
            for c in range(8, 12):
                emit_D(wTall[:, 16 + c, :], c, p5)
            emit_fp8_round(7)
            emit_wT_group(6)
            emit_wT_group(7)
            for c in range(12, 16):
                emit_D(wTall[:, 16 + c, :], c, p5)
            for j in range(16):
                emit_D(wd[:, j, :], j, pLL)

            # ---- output ----
            nc.scalar.copy(out_sb[:], out_ps[:])
            nc.sync.dma_start(outT[:, :], out_sb[:])

    nc.finalize()
    return nc


def _get_nc(use_bias: bool):
    if use_bias not in _BUILT:
        _BUILT[use_bias] = _build(use_bias)
    return _BUILT[use_bias]


def _make_in_maps(x, W, b, leaf_weight, gates):
    x = np.ascontiguousarray(np.asarray(x, dtype=np.float32))
    W = np.asarray(W, dtype=np.float32)
    b = np.asarray(b, dtype=np.float32)
    leaf_weight = np.asarray(leaf_weight, dtype=np.float32)
    gates = np.asarray(gates, dtype=np.float32)

    use_bias = bool(np.any(b))
    Wp = W[_NODES_PERM]                                   # [63, 512, 64] block order

    # bf16 banks: [node0 | -node0], then levels 1-4 (30 nodes, 2 per bank)
    bank0 = np.concatenate([Wp[0], -Wp[0]], axis=1)       # [512, 128]
    rest = Wp[1:31].transpose(1, 0, 2).reshape(D_IN, 30 * 64)
    allcols = np.concatenate([bank0, rest], axis=1)       # [512, 2048]
    Wbf = np.ascontiguousarray(
        allcols.reshape(KT, 128, NBF, 128).transpose(2, 1, 0, 3).astype(BF16))

    # fp8 level-5 stationaries: [bank(node pair), p, kpair, i, (node, t)]
    W8 = np.ascontiguousarray(
        Wp[31:63].reshape(2 * NF8, 2, 2, 2, 128, 64)      # [c, n, kp, i, p, t]
        .transpose(0, 4, 2, 3, 1, 5).reshape(2 * NF8, 128, 2, 2, 128).astype(FP8))

    gt = np.ascontiguousarray(
        gates[_LEAF_PERM].transpose(1, 0, 2).astype(BF16))     # [128, 64, 64]
    lwt = np.ascontiguousarray(
        leaf_weight[_LEAF_PERM].transpose(1, 0, 2).astype(BF16))

    if use_bias:
        bp = b[_NODES_PERM]                               # [63, 64]
        # 64-partition slots in phase-A emission order
        slots = np.concatenate(
            [np.concatenate([bp[0], -bp[0]]),             # bank 0: [b0 | -b0]
             bp[1:31].reshape(-1),                        # bf16 banks 1-15
             bp[31:63].reshape(-1)]).reshape(-1, 64)      # fp8: 32 lvl-5 nodes
        # bias column per sigmoid op: bf16 col m = bank m (slots 2m, 2m+1);
        # fp8 col NBF+2q+h = group q bank h (nodes 4q+2h, 4q+2h+1)
        biasd = np.zeros((128, NBF + 2 * NF8), np.float32)
        for m in range(NBF + 2 * NF8):
            biasd[0:64, m] = slots[2 * m]
            biasd[64:128, m] = slots[2 * m + 1]
        biasd = np.ascontiguousarray(biasd)

    in_maps = []
    for c in range(N_CORES):
        xs = x[c * BS:(c + 1) * BS]                       # [512, 512] (b, d)
        xdT = xs.T                                        # [512 d, 512 b]
        xTc = np.ascontiguousarray(
            xdT.reshape(KT, 128, BS).transpose(1, 0, 2).astype(BF16))
        x8c = np.ascontiguousarray(
            xdT.reshape(2, 2, 128, BS).transpose(2, 0, 1, 3).astype(FP8))
        m = {"xT": xTc, "x8": x8c, "Wbf": Wbf, "W8": W8, "gt": gt, "lwt": lwt}
        if use_bias:
            m["biasd"] = biasd
        in_maps.append(m)
    return use_bias, in_maps


def kernel(x, W, b, leaf_weight, gates):
    from concourse.bass_utils import run_bass_kernel_spmd

    use_bias, in_maps = _make_in_maps(x, W, b, leaf_weight, gates)
    nc = _get_nc(use_bias)

    res = run_bass_kernel_spmd(nc, in_maps, core_ids=list(range(N_CORES)))
    out = np.empty((BATCH, LEAF_DIMS), dtype=np.float32)
    for c in range(N_CORES):
        out[c * BS:(c + 1) * BS] = res.results[c]["outT"].T
    return out


# revision 50
# speedup vs baseline: 1.4841x; 1.4149x over previous
"""MixtureOfExpertsTreeEnsemble Trainium2 kernel (8-core SPMD, batch data-parallel).

Math (per batch row b, tree t):
  g[b,n,t] = sigmoid(x[b] @ W[n,:,t] + bias[n,t])          63 internal nodes
  p[b,l,t] = prod of g / (1-g) along root->leaf path        64 leaves
  w[l,d,t] = leaf_weight[l,d,t] * softmax_t(gates[l,d,t])
  out[b,d] = sum_{l,t} p[b,l,t] * w[l,d,t]

Sharding: batch 4096 -> 8 cores x 512 rows; node weights / leaf tables are
replicated.  No collectives; host concatenates the per-core outputs.

Device-side design (per core), all in a TRANSPOSED [(node,tree), batch]
layout so the path products need no PE transposes at all:

  * phase A (PE):   z^T[(n,t), b] tiles; nodes level-major, within-level
    bit-reversed ("block" order), trees inner.  Levels 0-4 (31 nodes) in
    bf16; level 5 (32 nodes) in fp8(e4m3) DoubleRow matmuls (2x PE rate;
    measured end-to-end rel-err ~1.3e-2 < 2e-2).  Node 0's bank holds
    [W0 | -W0] so one sigmoid op produces p1 = [g0 | 1-g0] for free.
  * sigmoid (ACT):  per 2-bank PSUM tile, writing per-level g tensors
    [128 part=(node,tree), 512 b].  exp for the softmax runs FIRST so the
    ACT function table loads only twice (exp set -> sigmoid set).
  * phase B (DVE):  level doubling entirely along partitions:
    p_{l+1} = [p_l * g_l | p_l * (1-g_l)]; the right half uses a fresh
    product with h=1-g (tensor_scalar) instead of a subtract to avoid a
    double-rounding that costs ~4e-3 of accuracy.
  * leaves:         only the LEFT leaf products pLL = p5*g5 materialize.
    The right-leaf term is folded into phase D algebraically:
      out = sum_j (wTL_j - wTR_j)^T pLL_j + sum_j wTR_j^T p5_j
    which deletes 32 DVE subtract ops at zero PE cost.
  * phase 0:        w = leaf_weight * softmax(gates): exp on ACT (bf16),
    tree-sum + recip on DVE, e*r broadcast on the otherwise idle Pool
    engine, *leaf_weight on DVE; PE transposes w -> [(leaf,tree), d].
  * phase D (PE):   out^T[d,b] accumulated over 32 [(l,t),*] chunks; the
    wT transposes and the first 16 chunk matmuls interleave into phase
    A-fp8's ACT-paced PE gaps.
  * DMA: weights + x on the SP HW-DGE ring, leaf tables on the ACT ring.
"""

import sys

sys.path.insert(0, "/opt/trn_rl_repo")

import ml_dtypes
import numpy as np

BF16 = np.dtype(ml_dtypes.bfloat16)
FP8 = np.dtype(ml_dtypes.float8_e4m3)

MAX_DEPTH = 6
NUM_TREES = 64
LEAF_DIMS = 128
D_IN = 512
BATCH = 4096
N_INTERNAL = 63
N_LEAVES = 64
N_CORES = 8
BS = BATCH // N_CORES          # 512 batch rows per core
KT = D_IN // 128               # 4 contraction tiles
NBF = 16                       # bf16 banks: lvl0+- 1, lvl1 1, lvl2 2, lvl3 4, lvl4 8
NF8 = 8                        # fp8 4-node groups (level 5: 32 nodes)


def _bitrev(x: int, bits: int) -> int:
    r = 0
    for _ in range(bits):
        r = (r << 1) | (x & 1)
        x >>= 1
    return r


# level-major, within-level bit-reversed (block recursion) node order
_NODES_PERM = np.array(
    [(2**lvl - 1) + _bitrev(j, lvl) for lvl in range(MAX_DEPTH) for j in range(2**lvl)]
)
_LEAF_PERM = np.array([_bitrev(j, MAX_DEPTH) for j in range(N_LEAVES)])

_BUILT = {}

_DMA_ORDER = ("gt1", "gt2", "xk", "Wb01", "Wb27", "Wb8F", "lwt", "x8", "W8a", "W8b")

# fp8-phase emission schedule: ("r", round) = fp8 matmul round + sigmoid +
# pLL; ("g", idx) = wT transpose group; ("c", c) = phase-D wTR chunk;
# ("d", j) = phase-D wd chunk.  Tuned against the timeline simulator.
_FP8_SCHED = (
    [("r", 0), ("r", 1), ("g", 0), ("r", 2), ("g", 1), ("r", 3), ("g", 2), ("g", 3)]
    + [("c", c) for c in range(0, 6)]
    + [("r", 4)]
    + [("c", c) for c in range(6, 12)]
    + [("r", 5)]
    + [("c", c) for c in range(12, 16)]
    + [("d", j) for j in range(0, 4)]
    + [("r", 6)]
    + [("d", j) for j in range(4, 8)]
    + [("r", 7)]
    + [("d", j) for j in range(8, 16)]
)


def _build(use_bias: bool):
    import concourse.bacc as bacc
    import concourse.tile as tile
    from concourse import mybir
    from concourse.masks import make_identity

    f32 = mybir.dt.float32
    bf16 = mybir.dt.bfloat16
    fp8 = mybir.dt.float8e4
    AF = mybir.ActivationFunctionType
    AX = mybir.AxisListType
    MUL = mybir.AluOpType.mult
    ADD = mybir.AluOpType.add
    DR = mybir.MatmulPerfMode.DoubleRow

    nc = bacc.Bacc("TRN2", target_bir_lowering=False, debug=False)

    xT = nc.dram_tensor("xT", [128, KT, BS], bf16, kind="ExternalInput")
    x8 = nc.dram_tensor("x8", [128, 2, 2, BS], fp8, kind="ExternalInput")
    Wbf = nc.dram_tensor("Wbf", [128, NBF, KT, 128], bf16, kind="ExternalInput")
    W8 = nc.dram_tensor("W8", [128, 2 * NF8, 2, 2, 128], fp8, kind="ExternalInput")
    gt = nc.dram_tensor("gt", [LEAF_DIMS, N_LEAVES, NUM_TREES], bf16, kind="ExternalInput")
    lwt = nc.dram_tensor("lwt", [LEAF_DIMS, N_LEAVES, NUM_TREES], bf16, kind="ExternalInput")
    if use_bias:
        biasd = nc.dram_tensor("biasd", [128, NBF + 2 * NF8], f32, kind="ExternalInput")
    outT = nc.dram_tensor("outT", [LEAF_DIMS, BS], f32, kind="ExternalOutput")

    with tile.TileContext(nc) as tc:
        with tc.tile_pool(name="const", bufs=1) as cpool, \
             tc.tile_pool(name="wts", bufs=1) as wpool, \
             tc.tile_pool(name="psA", bufs=2, space="PSUM") as psA, \
             tc.tile_pool(name="psT", bufs=3, space="PSUM") as psT, \
             tc.tile_pool(name="psO", bufs=1, space="PSUM") as psO:

            # ---- PE warm-up first: ramp the p-state with zero matmuls into
            # the (later restarted) output PSUM bank while DMAs land ----
            warm0 = cpool.tile([128, 128], bf16, tag="warm0")
            nc.gpsimd.memset(warm0[:], 0.0)
            out_ps = psO.tile([LEAF_DIMS, BS], f32, tag="out_ps")
            for _ in range(40):
                nc.tensor.matmul(out_ps[:, 0:128], warm0[:], warm0[:],
                                 start=True, stop=True)

            ident = cpool.tile([128, 128], bf16, tag="ident")
            make_identity(nc, ident[:])

            # ---- input DMAs.  The DMA engines drain one transfer at a time
            # (~340 GB/s), so everything goes on the SP ring in consumption-
            # priority order; only the final output uses the ACT ring (its
            # SEQ must stay free for exp/sigmoids). ----
            xk = wpool.tile([128, KT, BS], bf16, tag="xk")
            wbf_sb = wpool.tile([128, NBF, KT, 128], bf16, tag="wbf")
            gtile = wpool.tile([128, N_LEAVES, NUM_TREES], bf16, tag="gtile")
            lwtile = wpool.tile([128, N_LEAVES, NUM_TREES], bf16, tag="lwtile")
            x8sb = wpool.tile([128, 2, 2, BS], fp8, tag="x8sb")
            w8_sb = wpool.tile([128, 2 * NF8, 2, 2, 128], fp8, tag="w8")

            dma_emit = {
                "gt1": lambda: nc.sync.dma_start(gtile[:, 0:32, :], gt[:, 0:32, :]),
                "gt2": lambda: nc.sync.dma_start(gtile[:, 32:64, :], gt[:, 32:64, :]),
                "xk": lambda: nc.sync.dma_start(xk[:], xT[:, :, :]),
                "Wb01": lambda: nc.sync.dma_start(wbf_sb[:, 0:2, :, :], Wbf[:, 0:2, :, :]),
                "Wb27": lambda: nc.sync.dma_start(wbf_sb[:, 2:8, :, :], Wbf[:, 2:8, :, :]),
                "Wb8F": lambda: nc.sync.dma_start(wbf_sb[:, 8:16, :, :], Wbf[:, 8:16, :, :]),
                "lwt": lambda: nc.sync.dma_start(lwtile[:], lwt[:, :, :]),
                "x8": lambda: nc.sync.dma_start(x8sb[:], x8[:, :, :, :]),
                "W8a": lambda: nc.sync.dma_start(w8_sb[:, 0:8, :, :, :], W8[:, 0:8, :, :, :]),
                "W8b": lambda: nc.sync.dma_start(w8_sb[:, 8:16, :, :, :], W8[:, 8:16, :, :, :]),
            }
            for name in _DMA_ORDER:
                dma_emit[name]()
            if use_bias:
                bias_sb = cpool.tile([128, NBF + 2 * NF8], f32, tag="bias")
                nc.sync.dma_start(bias_sb[:], biasd[:, :])

            out_sb = wpool.tile([LEAF_DIMS, BS], f32, tag="out_sb")

            # ---- SBUF state ----
            g1 = wpool.tile([128, BS], bf16, tag="g1")
            g2 = wpool.tile([128, 2, BS], bf16, tag="g2")
            g3 = wpool.tile([128, 4, BS], bf16, tag="g3")
            g4 = wpool.tile([128, 8, BS], bf16, tag="g4")
            g5 = wpool.tile([128, 16, BS], bf16, tag="g5")
            h1 = wpool.tile([128, BS], bf16, tag="h1")
            h2 = wpool.tile([128, 2, BS], bf16, tag="h2")
            h3 = wpool.tile([128, 4, BS], bf16, tag="h3")
            h4 = wpool.tile([128, 8, BS], bf16, tag="h4")
            p1 = wpool.tile([128, BS], bf16, tag="p1")
            p2 = wpool.tile([128, 2, BS], bf16, tag="p2")
            p3 = wpool.tile([128, 4, BS], bf16, tag="p3")
            p4 = wpool.tile([128, 8, BS], bf16, tag="p4")
            p5 = wpool.tile([128, 16, BS], bf16, tag="p5")
            pLL = wpool.tile([128, 16, BS], bf16, tag="pLL")
            s_t = cpool.tile([128, N_LEAVES], bf16, tag="s_t")
            r_t = cpool.tile([128, N_LEAVES], bf16, tag="r_t")
            wsmt = wpool.tile([128, N_LEAVES, NUM_TREES], bf16, tag="wsmt")
            wTall = wpool.tile([128, 32, 128], bf16, tag="wTall")
            wd = wpool.tile([128, 16, 128], bf16, tag="wd")

            glv = [None, g1, g2, g3, g4, g5]
            hlv = [None, h1, h2, h3, h4]
            plv = [None, p1, p2, p3, p4, p5]

            # ---- ACT: softmax exp first (so the exp table load replaces the
            # initial sigmoid load; one switch to sigmoid afterwards) ----
            nc.scalar.activation(gtile[:, 0:32, :], gtile[:, 0:32, :], AF.Exp)
            nc.scalar.activation(gtile[:, 32:64, :], gtile[:, 32:64, :], AF.Exp)

            # ---- DVE/Pool softmax chain (emitted early; deps gate it) ----
            # tree-halving adds + short reduce per half on DVE (TensorReduce
            # runs in 1x mode, so halve twice in 2x mode first), then
            # en = e*r broadcast on Pool.  Slices pair the L (beta<32) and
            # matching R (beta>=32) ranges so each wT transpose group's
            # inputs complete together; the final wsm = en*lw DVE ops are
            # emitted later (interleaved into phase A) so they don't block
            # the in-order DVE path-product chain.
            eh1 = cpool.tile([128, 32, 32], bf16, tag="eh1")
            eh2 = cpool.tile([128, 32, 16], bf16, tag="eh2")
            with nc.allow_low_precision(reason="softmax denom in bf16: validated "
                                        "end-to-end rel-err impact < 5e-4"):
                for hh in range(2):
                    sl = slice(32 * hh, 32 * (hh + 1))
                    nc.vector.tensor_add(eh1[:], gtile[:, sl, 0:32],
                                         gtile[:, sl, 32:64])
                    nc.vector.tensor_add(eh2[:], eh1[:, :, 0:16], eh1[:, :, 16:32])
                    nc.vector.reduce_sum(s_t[:, sl], eh2[:], axis=AX.X)
                    nc.vector.reciprocal(r_t[:, sl], s_t[:, sl])
            wsm_slices = [slice(0, 16), slice(32, 48), slice(16, 32), slice(48, 64)]
            for sl in wsm_slices:
                rb = r_t[:, sl, None].broadcast_to((128, 16, NUM_TREES))
                nc.gpsimd.tensor_tensor(gtile[:, sl, :], gtile[:, sl, :], rb, op=MUL)

            def emit_wsm(idx):
                sl = wsm_slices[idx]
                nc.vector.tensor_mul(wsmt[:, sl, :], gtile[:, sl, :], lwtile[:, sl, :])

            # ---- helpers ----
            def sigmoid_op(src, dst, bias_col=None):
                if use_bias:
                    nc.scalar.activation(dst, src, AF.Sigmoid,
                                         bias=bias_sb[:, bias_col:bias_col + 1])
                else:
                    nc.scalar.activation(dst, src, AF.Sigmoid)

            def emit_bf16_tile(ti):
                """psA tile covering bf16 banks 2ti, 2ti+1 -> g tensors."""
                za = psA.tile([128, 2, BS], f32, tag="za")
                for hh in range(2):
                    m = 2 * ti + hh
                    for k in range(KT):
                        nc.tensor.matmul(za[:, hh, :], wbf_sb[:, m, k, :],
                                         xk[:, k, :], start=(k == 0), stop=(k == KT - 1))
                # sigmoid destinations
                if ti == 0:
                    sigmoid_op(za[:, 0, :], p1[:], 0)
                    sigmoid_op(za[:, 1, :], g1[:], 1)
                elif ti == 1:
                    if use_bias:
                        sigmoid_op(za[:, 0, :], g2[:, 0, :], 2)
                        sigmoid_op(za[:, 1, :], g2[:, 1, :], 3)
                    else:
                        sigmoid_op(za[:, :, :], g2[:, 0:2, :])
                else:
                    lvl = 3 if ti < 4 else 4
                    goff = 2 * (ti - 2) if ti < 4 else 2 * (ti - 4)
                    gdst = glv[lvl]
                    if use_bias:
                        sigmoid_op(za[:, 0, :], gdst[:, goff, :], 2 * ti)
                        sigmoid_op(za[:, 1, :], gdst[:, goff + 1, :], 2 * ti + 1)
                    else:
                        sigmoid_op(za[:, :, :], gdst[:, goff:goff + 2, :])

            def emit_fp8_round(r, with_pll=True):
                """4 level-5 nodes (banks 2r, 2r+1) in fp8 DoubleRow -> g5."""
                za = psA.tile([128, 2, BS], f32, tag="za")
                for hh in range(2):
                    c = 2 * r + hh
                    for bh in range(2):
                        for kp in range(2):
                            nc.tensor.matmul(
                                za[:, hh, bh * 256:(bh + 1) * 256],
                                w8_sb[:, c, kp, :, :],
                                x8sb[:, kp, :, bh * 256:(bh + 1) * 256],
                                start=(kp == 0), stop=(kp == 1), perf_mode=DR)
                if use_bias:
                    sigmoid_op(za[:, 0, :], g5[:, 2 * r, :], NBF + 2 * r)
                    sigmoid_op(za[:, 1, :], g5[:, 2 * r + 1, :], NBF + 2 * r + 1)
                else:
                    sigmoid_op(za[:, :, :], g5[:, 2 * r:2 * r + 2, :])
                if with_pll:
                    emit_pll(r)

            def emit_pll(r):
                csl = slice(2 * r, 2 * r + 2)
                nc.vector.tensor_mul(pLL[:, csl, :], p5[:, csl, :], g5[:, csl, :])

            def emit_level_products(lvl, coff, n):
                """p_{lvl+1} chunks [coff, coff+n) from p_lvl, g_lvl, h_lvl."""
                g, h, p, pn = glv[lvl], hlv[lvl], plv[lvl], plv[lvl + 1]
                half = pn.shape[1] // 2 if lvl > 1 else 1
                if lvl == 1:
                    nc.vector.tensor_scalar(h[:], g[:], -1.0, 1.0, op0=MUL, op1=ADD)
                    nc.vector.tensor_mul(pn[:, 0, :], p[:], g[:])
                    nc.vector.tensor_mul(pn[:, 1, :], p[:], h[:])
                else:
                    sl = slice(coff, coff + n)
                    slR = slice(half + coff, half + coff + n)
                    nc.vector.tensor_scalar(h[:, sl, :], g[:, sl, :], -1.0, 1.0,
                                            op0=MUL, op1=ADD)
                    nc.vector.tensor_mul(pn[:, sl, :], p[:, sl, :], g[:, sl, :])
                    nc.vector.tensor_mul(pn[:, slR, :], p[:, sl, :], h[:, sl, :])

            def emit_wT_group(gidx):
                """Transpose wsm chunks {4g..4g+3, 16+4g..16+4g+3}; DVE copies
                them out, Pool builds wd so DVE stays on path products."""
                tp = psT.tile([128, 8, 128], bf16, tag="tp")
                chunks = list(range(4 * gidx, 4 * gidx + 4)) + \
                    list(range(16 + 4 * gidx, 16 + 4 * gidx + 4))
                for qi, c in enumerate(chunks):
                    nc.tensor.transpose(tp[:, qi, :], wsmt[:, 2 * c:2 * c + 2, :], ident[:])
                nc.vector.tensor_copy(wTall[:, 4 * gidx:4 * gidx + 4, :], tp[:, 0:4, :])
                nc.vector.tensor_copy(wTall[:, 16 + 4 * gidx:16 + 4 * gidx + 4, :],
                                      tp[:, 4:8, :])
                nc.gpsimd.tensor_tensor(wd[:, 4 * gidx:4 * gidx + 4, :],
                                        wTall[:, 4 * gidx:4 * gidx + 4, :],
                                        wTall[:, 16 + 4 * gidx:16 + 4 * gidx + 4, :],
                                        op=mybir.AluOpType.subtract)

            dcount = [0]

            def emit_D(stationary, moving_chunk, moving):
                nc.tensor.matmul(out_ps[:], stationary, moving[:, moving_chunk, :],
                                 start=(dcount[0] == 0), stop=(dcount[0] == 31))
                dcount[0] += 1

            # ---- phase A bf16 (levels 0-4) with phase-B DVE ops interleaved ----
            emit_bf16_tile(0)                      # p1, g1
            emit_bf16_tile(1)                      # g2
            emit_level_products(1, 0, 1)           # p2
            emit_bf16_tile(2)                      # g3[0:2]
            emit_level_products(2, 0, 2)           # p3 (needs g2 only)
            emit_bf16_tile(3)                      # g3[2:4]
            emit_level_products(3, 0, 2)           # p4 chunks 0:2 / 4:6
            for i in range(4):                     # g4 tiles
                emit_bf16_tile(4 + i)
                if i == 0:
                    emit_level_products(3, 2, 2)   # rest of p4
                    emit_wsm(0)                    # enables wT groups 0-1
                    emit_wsm(1)
                emit_level_products(4, 2 * i, 2)   # p5 per g4 pair
            emit_wsm(2)                            # enables wT groups 2-3
            emit_wsm(3)

            # ---- phase A fp8 (level 5) + wT transposes + phase D interleave.
            # D-wTR chunks only need p5 + the wT copies; D-wd chunk j chases
            # pLL round j//2, so emit them staggered to keep PE off the tail.
            for kind, arg in _FP8_SCHED:
                if kind == "r":
                    emit_fp8_round(arg)
                elif kind == "g":
                    emit_wT_group(arg)
                elif kind == "c":
                    emit_D(wTall[:, 16 + arg, :], arg, p5)
                else:
                    emit_D(wd[:, arg, :], arg, pLL)

            # ---- output: halves on separate rings so the two DMA chains
            # (descriptor gen + transfer + completion) overlap ----
            nc.scalar.copy(out_sb[:, 0:256], out_ps[:, 0:256])
            nc.sync.dma_start(outT[:, 0:256], out_sb[:, 0:256])
            nc.scalar.copy(out_sb[:, 256:512], out_ps[:, 256:512])
            nc.scalar.dma_start(outT[:, 256:512], out_sb[:, 256:512])

    nc.finalize()
    return nc


def _get_nc(use_bias: bool):
    if use_bias not in _BUILT:
        _BUILT[use_bias] = _build(use_bias)
    return _BUILT[use_bias]


def _make_in_maps(x, W, b, leaf_weight, gates):
    x = np.ascontiguousarray(np.asarray(x, dtype=np.float32))
    W = np.asarray(W, dtype=np.float32)
    b = np.asarray(b, dtype=np.float32)
    leaf_weight = np.asarray(leaf_weight, dtype=np.float32)
    gates = np.asarray(gates, dtype=np.float32)

    use_bias = bool(np.any(b))
    Wp = W[_NODES_PERM]                                   # [63, 512, 64] block order

    # bf16 banks: [node0 | -node0], then levels 1-4 (30 nodes, 2 per bank)
    bank0 = np.concatenate([Wp[0], -Wp[0]], axis=1)       # [512, 128]
    rest = Wp[1:31].transpose(1, 0, 2).reshape(D_IN, 30 * 64)
    allcols = np.concatenate([bank0, rest], axis=1)       # [512, 2048]
    Wbf = np.ascontiguousarray(
        allcols.reshape(KT, 128, NBF, 128).transpose(1, 2, 0, 3).astype(BF16))

    # fp8 level-5 stationaries: [p, bank(node pair), kpair, i, (node, t)]
    W8 = np.ascontiguousarray(
        Wp[31:63].reshape(2 * NF8, 2, 2, 2, 128, 64)      # [c, n, kp, i, p, t]
        .transpose(4, 0, 2, 3, 1, 5).reshape(128, 2 * NF8, 2, 2, 128).astype(FP8))

    gt = np.ascontiguousarray(
        gates[_LEAF_PERM].transpose(1, 0, 2).astype(BF16))     # [128, 64, 64]
    lwt = np.ascontiguousarray(
        leaf_weight[_LEAF_PERM].transpose(1, 0, 2).astype(BF16))

    if use_bias:
        bp = b[_NODES_PERM]                               # [63, 64]
        # 64-partition slots in phase-A emission order
        slots = np.concatenate(
            [np.concatenate([bp[0], -bp[0]]),             # bank 0: [b0 | -b0]
             bp[1:31].reshape(-1),                        # bf16 banks 1-15
             bp[31:63].reshape(-1)]).reshape(-1, 64)      # fp8: 32 lvl-5 nodes
        # bias column per sigmoid op: bf16 col m = bank m (slots 2m, 2m+1);
        # fp8 col NBF+2q+h = group q bank h (nodes 4q+2h, 4q+2h+1)
        biasd = np.zeros((128, NBF + 2 * NF8), np.float32)
        for m in range(NBF + 2 * NF8):
            biasd[0:64, m] = slots[2 * m]
            biasd[64:128, m] = slots[2 * m + 1]
        biasd = np.ascontiguousarray(biasd)

    in_maps = []
    for c in range(N_CORES):
        xs = x[c * BS:(c + 1) * BS]                       # [512, 512] (b, d)
        xdT = xs.T                                        # [512 d, 512 b]
        xTc = np.ascontiguousarray(
            xdT.reshape(KT, 128, BS).transpose(1, 0, 2).astype(BF16))
        x8c = np.ascontiguousarray(
            xdT.reshape(2, 2, 128, BS).transpose(2, 0, 1, 3).astype(FP8))
        m = {"xT": xTc, "x8": x8c, "Wbf": Wbf, "W8": W8, "gt": gt, "lwt": lwt}
        if use_bias:
            m["biasd"] = biasd
        in_maps.append(m)
    return use_bias, in_maps


def kernel(x, W, b, leaf_weight, gates):
    from concourse.bass_utils import run_bass_kernel_spmd

    use_bias, in_maps = _make_in_maps(x, W, b, leaf_weight, gates)
    nc = _get_nc(use_bias)

    res = run_bass_kernel_spmd(nc, in_maps, core_ids=list(range(N_CORES)))
    out = np.empty((BATCH, LEAF_DIMS), dtype=np.float32)
    for c in range(N_CORES):
        out[c * BS:(c + 1) * BS] = res.results[c]["outT"].T
    return out


# revision 51
# speedup vs baseline: 1.4877x; 1.0024x over previous
"""MixtureOfExpertsTreeEnsemble Trainium2 kernel (8-core SPMD, batch data-parallel).

Math (per batch row b, tree t):
  g[b,n,t] = sigmoid(x[b] @ W[n,:,t] + bias[n,t])          63 internal nodes
  p[b,l,t] = prod of g / (1-g) along root->leaf path        64 leaves
  w[l,d,t] = leaf_weight[l,d,t] * softmax_t(gates[l,d,t])
  out[b,d] = sum_{l,t} p[b,l,t] * w[l,d,t]

Sharding: batch 4096 -> 8 cores x 512 rows; node weights / leaf tables are
replicated.  No collectives; host concatenates the per-core outputs.

Device-side design (per core), all in a TRANSPOSED [(node,tree), batch]
layout so the path products need no PE transposes at all:

  * phase A (PE):   z^T[(n,t), b] tiles; nodes level-major, within-level
    bit-reversed ("block" order), trees inner.  Levels 0-4 (31 nodes) in
    bf16; level 5 (32 nodes) in fp8(e4m3) DoubleRow matmuls (2x PE rate;
    measured end-to-end rel-err ~1.3e-2 < 2e-2).  Node 0's bank holds
    [W0 | -W0] so one sigmoid op produces p1 = [g0 | 1-g0] for free.
  * sigmoid (ACT):  per 2-bank PSUM tile, writing per-level g tensors
    [128 part=(node,tree), 512 b].  exp for the softmax runs FIRST so the
    ACT function table loads only twice (exp set -> sigmoid set).
  * phase B (DVE):  level doubling entirely along partitions:
    p_{l+1} = [p_l * g_l | p_l * (1-g_l)]; the right half uses a fresh
    product with h=1-g (tensor_scalar) instead of a subtract to avoid a
    double-rounding that costs ~4e-3 of accuracy.
  * leaves:         only the LEFT leaf products pLL = p5*g5 materialize.
    The right-leaf term is folded into phase D algebraically:
      out = sum_j (wTL_j - wTR_j)^T pLL_j + sum_j wTR_j^T p5_j
    which deletes 32 DVE subtract ops at zero PE cost.
  * phase 0:        w = leaf_weight * softmax(gates): exp on ACT (bf16),
    tree-sum + recip on DVE, e*r broadcast on the otherwise idle Pool
    engine, *leaf_weight on DVE; PE transposes w -> [(leaf,tree), d].
  * phase D (PE):   out^T[d,b] accumulated over 32 [(l,t),*] chunks; the
    wT transposes and the first 16 chunk matmuls interleave into phase
    A-fp8's ACT-paced PE gaps.
  * DMA: weights + x on the SP HW-DGE ring, leaf tables on the ACT ring.
"""

import sys

sys.path.insert(0, "/opt/trn_rl_repo")

import ml_dtypes
import numpy as np

BF16 = np.dtype(ml_dtypes.bfloat16)
FP8 = np.dtype(ml_dtypes.float8_e4m3)

MAX_DEPTH = 6
NUM_TREES = 64
LEAF_DIMS = 128
D_IN = 512
BATCH = 4096
N_INTERNAL = 63
N_LEAVES = 64
N_CORES = 8
BS = BATCH // N_CORES          # 512 batch rows per core
KT = D_IN // 128               # 4 contraction tiles
NBF = 16                       # bf16 banks: lvl0+- 1, lvl1 1, lvl2 2, lvl3 4, lvl4 8
NF8 = 8                        # fp8 4-node groups (level 5: 32 nodes)


def _bitrev(x: int, bits: int) -> int:
    r = 0
    for _ in range(bits):
        r = (r << 1) | (x & 1)
        x >>= 1
    return r


# level-major, within-level bit-reversed (block recursion) node order
_NODES_PERM = np.array(
    [(2**lvl - 1) + _bitrev(j, lvl) for lvl in range(MAX_DEPTH) for j in range(2**lvl)]
)
_LEAF_PERM = np.array([_bitrev(j, MAX_DEPTH) for j in range(N_LEAVES)])

_BUILT = {}

_DMA_ORDER = ("gt1", "gt2", "xk", "Wb01", "Wb27", "Wb8F", "lwt", "x8", "W8a", "W8b")

# fp8-phase emission schedule: ("r", round) = fp8 matmul round + sigmoid +
# pLL; ("g", idx) = wT transpose group; ("c", c) = phase-D wTR chunk;
# ("d", j) = phase-D wd chunk.  Tuned against the timeline simulator.
_FP8_SCHED = (
    [("r", 0), ("r", 1), ("g", 0), ("r", 2), ("g", 1), ("r", 3), ("g", 2), ("g", 3)]
    + [("c", c) for c in range(0, 6)]
    + [("r", 4)]
    + [("c", c) for c in range(6, 12)]
    + [("r", 5)]
    + [("c", c) for c in range(12, 16)]
    + [("d", j) for j in range(0, 4)]
    + [("r", 6)]
    + [("d", j) for j in range(4, 8)]
    + [("r", 7)]
    + [("d", j) for j in range(8, 16)]
)


def _build(use_bias: bool):
    import concourse.bacc as bacc
    import concourse.tile as tile
    from concourse import mybir
    from concourse.masks import make_identity

    f32 = mybir.dt.float32
    bf16 = mybir.dt.bfloat16
    fp8 = mybir.dt.float8e4
    AF = mybir.ActivationFunctionType
    AX = mybir.AxisListType
    MUL = mybir.AluOpType.mult
    ADD = mybir.AluOpType.add
    DR = mybir.MatmulPerfMode.DoubleRow

    nc = bacc.Bacc("TRN2", target_bir_lowering=False, debug=False)

    xT = nc.dram_tensor("xT", [128, KT, BS], bf16, kind="ExternalInput")
    x8 = nc.dram_tensor("x8", [128, 2, 2, BS], fp8, kind="ExternalInput")
    Wbf = nc.dram_tensor("Wbf", [128, NBF, KT, 128], bf16, kind="ExternalInput")
    W8 = nc.dram_tensor("W8", [128, 2 * NF8, 2, 2, 128], fp8, kind="ExternalInput")
    gt = nc.dram_tensor("gt", [LEAF_DIMS, N_LEAVES, NUM_TREES], bf16, kind="ExternalInput")
    lwt = nc.dram_tensor("lwt", [LEAF_DIMS, N_LEAVES, NUM_TREES], bf16, kind="ExternalInput")
    if use_bias:
        biasd = nc.dram_tensor("biasd", [128, NBF + 2 * NF8], f32, kind="ExternalInput")
    outT = nc.dram_tensor("outT", [LEAF_DIMS, BS], f32, kind="ExternalOutput")

    with tile.TileContext(nc) as tc:
        with tc.tile_pool(name="const", bufs=1) as cpool, \
             tc.tile_pool(name="wts", bufs=1) as wpool, \
             tc.tile_pool(name="psA", bufs=2, space="PSUM") as psA, \
             tc.tile_pool(name="psT", bufs=3, space="PSUM") as psT, \
             tc.tile_pool(name="psO", bufs=1, space="PSUM") as psO:

            # ---- PE warm-up first: ramp the p-state with zero matmuls into
            # the (later restarted) output PSUM bank while DMAs land ----
            warm0 = cpool.tile([128, 128], bf16, tag="warm0")
            nc.gpsimd.memset(warm0[:], 0.0)
            out_ps = psO.tile([LEAF_DIMS, BS], f32, tag="out_ps")
            for _ in range(40):
                nc.tensor.matmul(out_ps[:, 0:128], warm0[:], warm0[:],
                                 start=True, stop=True)

            ident = cpool.tile([128, 128], bf16, tag="ident")
            make_identity(nc, ident[:])

            # ---- input DMAs.  The DMA engines drain one transfer at a time
            # (~340 GB/s), so everything goes on the SP ring in consumption-
            # priority order; only the final output uses the ACT ring (its
            # SEQ must stay free for exp/sigmoids). ----
            xk = wpool.tile([128, KT, BS], bf16, tag="xk")
            wbf_sb = wpool.tile([128, NBF, KT, 128], bf16, tag="wbf")
            gtile = wpool.tile([128, N_LEAVES, NUM_TREES], bf16, tag="gtile")
            lwtile = wpool.tile([128, N_LEAVES, NUM_TREES], bf16, tag="lwtile")
            x8sb = wpool.tile([128, 2, 2, BS], fp8, tag="x8sb")
            w8_sb = wpool.tile([128, 2 * NF8, 2, 2, 128], fp8, tag="w8")

            dma_emit = {
                "gt1": lambda: [nc.sync.dma_start(gtile[:, 16 * q:16 * (q + 1), :],
                                                  gt[:, 16 * q:16 * (q + 1), :])
                                for q in range(2)],
                "gt2": lambda: [nc.sync.dma_start(gtile[:, 16 * q:16 * (q + 1), :],
                                                  gt[:, 16 * q:16 * (q + 1), :])
                                for q in range(2, 4)],
                "xk": lambda: nc.sync.dma_start(xk[:], xT[:, :, :]),
                "Wb01": lambda: nc.sync.dma_start(wbf_sb[:, 0:2, :, :], Wbf[:, 0:2, :, :]),
                "Wb27": lambda: nc.sync.dma_start(wbf_sb[:, 2:8, :, :], Wbf[:, 2:8, :, :]),
                "Wb8F": lambda: nc.sync.dma_start(wbf_sb[:, 8:16, :, :], Wbf[:, 8:16, :, :]),
                "lwt": lambda: nc.sync.dma_start(lwtile[:], lwt[:, :, :]),
                "x8": lambda: nc.sync.dma_start(x8sb[:], x8[:, :, :, :]),
                "W8a": lambda: nc.sync.dma_start(w8_sb[:, 0:8, :, :, :], W8[:, 0:8, :, :, :]),
                "W8b": lambda: nc.sync.dma_start(w8_sb[:, 8:16, :, :, :], W8[:, 8:16, :, :, :]),
            }
            for name in _DMA_ORDER:
                dma_emit[name]()
            if use_bias:
                bias_sb = cpool.tile([128, NBF + 2 * NF8], f32, tag="bias")
                nc.sync.dma_start(bias_sb[:], biasd[:, :])

            out_sb = wpool.tile([LEAF_DIMS, BS], f32, tag="out_sb")

            # ---- SBUF state ----
            g1 = wpool.tile([128, BS], bf16, tag="g1")
            g2 = wpool.tile([128, 2, BS], bf16, tag="g2")
            g3 = wpool.tile([128, 4, BS], bf16, tag="g3")
            g4 = wpool.tile([128, 8, BS], bf16, tag="g4")
            g5 = wpool.tile([128, 16, BS], bf16, tag="g5")
            h1 = wpool.tile([128, BS], bf16, tag="h1")
            h2 = wpool.tile([128, 2, BS], bf16, tag="h2")
            h3 = wpool.tile([128, 4, BS], bf16, tag="h3")
            h4 = wpool.tile([128, 8, BS], bf16, tag="h4")
            p1 = wpool.tile([128, BS], bf16, tag="p1")
            p2 = wpool.tile([128, 2, BS], bf16, tag="p2")
            p3 = wpool.tile([128, 4, BS], bf16, tag="p3")
            p4 = wpool.tile([128, 8, BS], bf16, tag="p4")
            p5 = wpool.tile([128, 16, BS], bf16, tag="p5")
            pLL = wpool.tile([128, 16, BS], bf16, tag="pLL")
            s_t = cpool.tile([128, N_LEAVES], bf16, tag="s_t")
            r_t = cpool.tile([128, N_LEAVES], bf16, tag="r_t")
            wsmt = wpool.tile([128, N_LEAVES, NUM_TREES], bf16, tag="wsmt")
            wTall = wpool.tile([128, 32, 128], bf16, tag="wTall")
            wd = wpool.tile([128, 16, 128], bf16, tag="wd")

            glv = [None, g1, g2, g3, g4, g5]
            hlv = [None, h1, h2, h3, h4]
            plv = [None, p1, p2, p3, p4, p5]

            # ---- ACT: softmax exp first (so the exp table load replaces the
            # initial sigmoid load; one switch to sigmoid afterwards) ----
            for q in range(4):
                sl = slice(16 * q, 16 * (q + 1))
                nc.scalar.activation(gtile[:, sl, :], gtile[:, sl, :], AF.Exp)

            # ---- DVE/Pool softmax chain (emitted early; deps gate it) ----
            # tree-halving adds + short reduce per half on DVE (TensorReduce
            # runs in 1x mode, so halve twice in 2x mode first), then
            # en = e*r broadcast on Pool.  Slices pair the L (beta<32) and
            # matching R (beta>=32) ranges so each wT transpose group's
            # inputs complete together; the final wsm = en*lw DVE ops are
            # emitted later (interleaved into phase A) so they don't block
            # the in-order DVE path-product chain.
            eh1 = cpool.tile([128, 32, 32], bf16, tag="eh1")
            eh2 = cpool.tile([128, 32, 16], bf16, tag="eh2")
            with nc.allow_low_precision(reason="softmax denom in bf16: validated "
                                        "end-to-end rel-err impact < 5e-4"):
                for hh in range(2):
                    sl = slice(32 * hh, 32 * (hh + 1))
                    nc.vector.tensor_add(eh1[:], gtile[:, sl, 0:32],
                                         gtile[:, sl, 32:64])
                    nc.vector.tensor_add(eh2[:], eh1[:, :, 0:16], eh1[:, :, 16:32])
                    nc.vector.reduce_sum(s_t[:, sl], eh2[:], axis=AX.X)
                    nc.vector.reciprocal(r_t[:, sl], s_t[:, sl])
            wsm_slices = [slice(0, 16), slice(32, 48), slice(16, 32), slice(48, 64)]
            for sl in wsm_slices:
                rb = r_t[:, sl, None].broadcast_to((128, 16, NUM_TREES))
                nc.gpsimd.tensor_tensor(gtile[:, sl, :], gtile[:, sl, :], rb, op=MUL)

            def emit_wsm(idx):
                sl = wsm_slices[idx]
                nc.vector.tensor_mul(wsmt[:, sl, :], gtile[:, sl, :], lwtile[:, sl, :])

            # ---- helpers ----
            def sigmoid_op(src, dst, bias_col=None):
                if use_bias:
                    nc.scalar.activation(dst, src, AF.Sigmoid,
                                         bias=bias_sb[:, bias_col:bias_col + 1])
                else:
                    nc.scalar.activation(dst, src, AF.Sigmoid)

            def emit_bf16_tile(ti):
                """psA tile covering bf16 banks 2ti, 2ti+1 -> g tensors."""
                za = psA.tile([128, 2, BS], f32, tag="za")
                for hh in range(2):
                    m = 2 * ti + hh
                    for k in range(KT):
                        nc.tensor.matmul(za[:, hh, :], wbf_sb[:, m, k, :],
                                         xk[:, k, :], start=(k == 0), stop=(k == KT - 1))
                # sigmoid destinations
                if ti == 0:
                    sigmoid_op(za[:, 0, :], p1[:], 0)
                    sigmoid_op(za[:, 1, :], g1[:], 1)
                elif ti == 1:
                    if use_bias:
                        sigmoid_op(za[:, 0, :], g2[:, 0, :], 2)
                        sigmoid_op(za[:, 1, :], g2[:, 1, :], 3)
                    else:
                        sigmoid_op(za[:, :, :], g2[:, 0:2, :])
                else:
                    lvl = 3 if ti < 4 else 4
                    goff = 2 * (ti - 2) if ti < 4 else 2 * (ti - 4)
                    gdst = glv[lvl]
                    if use_bias:
                        sigmoid_op(za[:, 0, :], gdst[:, goff, :], 2 * ti)
                        sigmoid_op(za[:, 1, :], gdst[:, goff + 1, :], 2 * ti + 1)
                    else:
                        sigmoid_op(za[:, :, :], gdst[:, goff:goff + 2, :])

            def emit_fp8_round(r, with_pll=True):
                """4 level-5 nodes (banks 2r, 2r+1) in fp8 DoubleRow -> g5."""
                za = psA.tile([128, 2, BS], f32, tag="za")
                for hh in range(2):
                    c = 2 * r + hh
                    for bh in range(2):
                        for kp in range(2):
                            nc.tensor.matmul(
                                za[:, hh, bh * 256:(bh + 1) * 256],
                                w8_sb[:, c, kp, :, :],
                                x8sb[:, kp, :, bh * 256:(bh + 1) * 256],
                                start=(kp == 0), stop=(kp == 1), perf_mode=DR)
                if use_bias:
                    sigmoid_op(za[:, 0, :], g5[:, 2 * r, :], NBF + 2 * r)
                    sigmoid_op(za[:, 1, :], g5[:, 2 * r + 1, :], NBF + 2 * r + 1)
                else:
                    sigmoid_op(za[:, :, :], g5[:, 2 * r:2 * r + 2, :])
                if with_pll:
                    emit_pll(r)

            def emit_pll(r):
                csl = slice(2 * r, 2 * r + 2)
                nc.vector.tensor_mul(pLL[:, csl, :], p5[:, csl, :], g5[:, csl, :])

            def emit_level_products(lvl, coff, n):
                """p_{lvl+1} chunks [coff, coff+n) from p_lvl, g_lvl, h_lvl."""
                g, h, p, pn = glv[lvl], hlv[lvl], plv[lvl], plv[lvl + 1]
                half = pn.shape[1] // 2 if lvl > 1 else 1
                if lvl == 1:
                    nc.vector.tensor_scalar(h[:], g[:], -1.0, 1.0, op0=MUL, op1=ADD)
                    nc.vector.tensor_mul(pn[:, 0, :], p[:], g[:])
                    nc.vector.tensor_mul(pn[:, 1, :], p[:], h[:])
                else:
                    sl = slice(coff, coff + n)
                    slR = slice(half + coff, half + coff + n)
                    nc.vector.tensor_scalar(h[:, sl, :], g[:, sl, :], -1.0, 1.0,
                                            op0=MUL, op1=ADD)
                    nc.vector.tensor_mul(pn[:, sl, :], p[:, sl, :], g[:, sl, :])
                    nc.vector.tensor_mul(pn[:, slR, :], p[:, sl, :], h[:, sl, :])

            def emit_wT_group(gidx):
                """Transpose wsm chunks {4g..4g+3, 16+4g..16+4g+3}; DVE copies
                them out, Pool builds wd so DVE stays on path products."""
                tp = psT.tile([128, 8, 128], bf16, tag="tp")
                chunks = list(range(4 * gidx, 4 * gidx + 4)) + \
                    list(range(16 + 4 * gidx, 16 + 4 * gidx + 4))
                for qi, c in enumerate(chunks):
                    nc.tensor.transpose(tp[:, qi, :], wsmt[:, 2 * c:2 * c + 2, :], ident[:])
                nc.vector.tensor_copy(wTall[:, 4 * gidx:4 * gidx + 4, :], tp[:, 0:4, :])
                nc.vector.tensor_copy(wTall[:, 16 + 4 * gidx:16 + 4 * gidx + 4, :],
                                      tp[:, 4:8, :])
                nc.gpsimd.tensor_tensor(wd[:, 4 * gidx:4 * gidx + 4, :],
                                        wTall[:, 4 * gidx:4 * gidx + 4, :],
                                        wTall[:, 16 + 4 * gidx:16 + 4 * gidx + 4, :],
                                        op=mybir.AluOpType.subtract)

            dcount = [0]

            def emit_D(stationary, moving_chunk, moving):
                nc.tensor.matmul(out_ps[:], stationary, moving[:, moving_chunk, :],
                                 start=(dcount[0] == 0), stop=(dcount[0] == 31))
                dcount[0] += 1

            # ---- phase A bf16 (levels 0-4) with phase-B DVE ops interleaved ----
            emit_bf16_tile(0)                      # p1, g1
            emit_bf16_tile(1)                      # g2
            emit_level_products(1, 0, 1)           # p2
            emit_bf16_tile(2)                      # g3[0:2]
            emit_level_products(2, 0, 2)           # p3 (needs g2 only)
            emit_bf16_tile(3)                      # g3[2:4]
            emit_level_products(3, 0, 2)           # p4 chunks 0:2 / 4:6
            for i in range(4):                     # g4 tiles
                emit_bf16_tile(4 + i)
                if i == 0:
                    emit_level_products(3, 2, 2)   # rest of p4
                    emit_wsm(0)                    # enables wT groups 0-1
                    emit_wsm(1)
                emit_level_products(4, 2 * i, 2)   # p5 per g4 pair
            emit_wsm(2)                            # enables wT groups 2-3
            emit_wsm(3)

            # ---- phase A fp8 (level 5) + wT transposes + phase D interleave.
            # D-wTR chunks only need p5 + the wT copies; D-wd chunk j chases
            # pLL round j//2, so emit them staggered to keep PE off the tail.
            for kind, arg in _FP8_SCHED:
                if kind == "r":
                    emit_fp8_round(arg)
                elif kind == "g":
                    emit_wT_group(arg)
                elif kind == "c":
                    emit_D(wTall[:, 16 + arg, :], arg, p5)
                else:
                    emit_D(wd[:, arg, :], arg, pLL)

            # ---- output: halves on separate rings so the two DMA chains
            # (descriptor gen + transfer + completion) overlap ----
            nc.scalar.copy(out_sb[:, 0:256], out_ps[:, 0:256])
            nc.sync.dma_start(outT[:, 0:256], out_sb[:, 0:256])
            nc.scalar.copy(out_sb[:, 256:512], out_ps[:, 256:512])
            nc.scalar.dma_start(outT[:, 256:512], out_sb[:, 256:512])

    nc.finalize()
    return nc


def _get_nc(use_bias: bool):
    if use_bias not in _BUILT:
        _BUILT[use_bias] = _build(use_bias)
    return _BUILT[use_bias]


def _make_in_maps(x, W, b, leaf_weight, gates):
    x = np.ascontiguousarray(np.asarray(x, dtype=np.float32))
    W = np.asarray(W, dtype=np.float32)
    b = np.asarray(b, dtype=np.float32)
    leaf_weight = np.asarray(leaf_weight, dtype=np.float32)
    gates = np.asarray(gates, dtype=np.float32)

    use_bias = bool(np.any(b))
    Wp = W[_NODES_PERM]                                   # [63, 512, 64] block order

    # bf16 banks: [node0 | -node0], then levels 1-4 (30 nodes, 2 per bank)
    bank0 = np.concatenate([Wp[0], -Wp[0]], axis=1)       # [512, 128]
    rest = Wp[1:31].transpose(1, 0, 2).reshape(D_IN, 30 * 64)
    allcols = np.concatenate([bank0, rest], axis=1)       # [512, 2048]
    Wbf = np.ascontiguousarray(
        allcols.reshape(KT, 128, NBF, 128).transpose(1, 2, 0, 3).astype(BF16))

    # fp8 level-5 stationaries: [p, bank(node pair), kpair, i, (node, t)]
    W8 = np.ascontiguousarray(
        Wp[31:63].reshape(2 * NF8, 2, 2, 2, 128, 64)      # [c, n, kp, i, p, t]
        .transpose(4, 0, 2, 3, 1, 5).reshape(128, 2 * NF8, 2, 2, 128).astype(FP8))

    gt = np.ascontiguousarray(
        gates[_LEAF_PERM].transpose(1, 0, 2).astype(BF16))     # [128, 64, 64]
    lwt = np.ascontiguousarray(
        leaf_weight[_LEAF_PERM].transpose(1, 0, 2).astype(BF16))

    if use_bias:
        bp = b[_NODES_PERM]                               # [63, 64]
        # 64-partition slots in phase-A emission order
        slots = np.concatenate(
            [np.concatenate([bp[0], -bp[0]]),             # bank 0: [b0 | -b0]
             bp[1:31].reshape(-1),                        # bf16 banks 1-15
             bp[31:63].reshape(-1)]).reshape(-1, 64)      # fp8: 32 lvl-5 nodes
        # bias column per sigmoid op: bf16 col m = bank m (slots 2m, 2m+1);
        # fp8 col NBF+2q+h = group q bank h (nodes 4q+2h, 4q+2h+1)
        biasd = np.zeros((128, NBF + 2 * NF8), np.float32)
        for m in range(NBF + 2 * NF8):
            biasd[0:64, m] = slots[2 * m]
            biasd[64:128, m] = slots[2 * m + 1]
        biasd = np.ascontiguousarray(biasd)

    in_maps = []
    for c in range(N_CORES):
        xs = x[c * BS:(c + 1) * BS]                       # [512, 512] (b, d)
        xdT = xs.T                                        # [512 d, 512 b]
        xTc = np.ascontiguousarray(
            xdT.reshape(KT, 128, BS).transpose(1, 0, 2).astype(BF16))
        x8c = np.ascontiguousarray(
            xdT.reshape(2, 2, 128, BS).transpose(2, 0, 1, 3).astype(FP8))
        m = {"xT": xTc, "x8": x8c, "Wbf": Wbf, "W8": W8, "gt": gt, "lwt": lwt}
        if use_bias:
            m["biasd"] = biasd
        in_maps.append(m)
    return use_bias, in_maps


def kernel(x, W, b, leaf_weight, gates):
    from concourse.bass_utils import run_bass_kernel_spmd

    use_bias, in_maps = _make_in_maps(x, W, b, leaf_weight, gates)
    nc = _get_nc(use_bias)

    res = run_bass_kernel_spmd(nc, in_maps, core_ids=list(range(N_CORES)))
    out = np.empty((BATCH, LEAF_DIMS), dtype=np.float32)
    for c in range(N_CORES):
        out[c * BS:(c + 1) * BS] = res.results[c]["outT"].T
    return out
